# revision 40
# baseline (speedup 1.0000x reference)
"""Trainium2 Bass kernel for nn_Attention_23424751632639.

Computation (per (b,h)):  out = tril_strict(rope(Q) @ rope(Q).T / sqrt(N)) @ V
Reformulated as chunked linear attention (exact, just reordered sums):
  out_c = QR_c @ M_c  +  strict_mask(QR_c @ QR_c^T) @ V_c
  M_{c+1} = M_c + QR_c^T @ V_c            (M is the [64,64] running state)
with QR = rope(Q) * N**-0.25 (scale folded into the cos/sin tables, so the
score scale N**-0.5 appears automatically in both the intra and inter terms).

RoPE is computed as  QR = Q*CC + swap(Q)*SS  where swap exchanges feature
pairs (2m <-> 2m+1) and the rotation sign is folded into SS.  swap runs on
GpSimd (otherwise idle), the three wide elementwise ops on DVE.

Matmul operands are bf16 (PE 1 cyc/row vs 4 for fp32); all accumulation
(PSUM, the M state) stays fp32.  QR^T strips come from PE transposes.
Chunks are processed in pairs sharing PSUM tiles so each DVE/ACT fixup op
runs once per pair at double width.

Sharding: B*H = 32 (b,h) pairs -> 4 per core across 8 cores; no collectives.
"""

import math
import sys

import numpy as np

if "/opt/trn_rl_repo" not in sys.path:
    sys.path.insert(0, "/opt/trn_rl_repo")

B, H, T, N = 2, 16, 4096, 64
THETA = 2.0 ** 16
NCORES = 8
HPC = (B * H) // NCORES  # heads per core
QTR = 8  # chunks per pipeline stage (quarter-head granularity)


def _host_tables(t_len):
    """Full-width scaled RoPE tables CC, SS [t_len, N] float32.

    QR[t,n] = Q[t,n]*CC[t,n] + swap(Q)[t,n]*SS[t,n]
    where swap(Q)[2m] = Q[2m+1], swap(Q)[2m+1] = Q[2m]; the rotation minus
    sign lives in SS's even columns.
    """
    n = np.arange(N, dtype=np.float64)
    tq = np.floor(n / 2.0) * 2.0
    freqs = 1.0 / (THETA ** (tq / N)) / (2.0 * math.pi)  # [N]
    t = np.arange(t_len, dtype=np.float64)[:, None]
    ang = ((t * freqs[None, :]) % 1.0) * (2.0 * math.pi)  # [t_len, N]
    scale = float(N) ** -0.25
    cc = (np.cos(ang) * scale).astype(np.float32)
    ss = (np.sin(ang) * scale).astype(np.float32)
    ss[:, 0::2] *= -1.0
    return np.ascontiguousarray(cc), np.ascontiguousarray(ss)


def build_program(t_len=T, hpc=HPC):
    import concourse.mybir as mybir
    import concourse.tile as tile
    from concourse import bacc

    f32 = mybir.dt.float32
    bf = mybir.dt.bfloat16
    ch = t_len // 128  # number of 128-row chunks per head
    qtr = min(QTR, ch)  # chunks per pipeline stage
    nq = ch // qtr  # pipeline stages per head

    nc = bacc.Bacc(None, target_bir_lowering=False)
    q = nc.dram_tensor("q", [hpc, t_len, N], f32, kind="ExternalInput")
    v = nc.dram_tensor("v", [hpc, t_len, N], f32, kind="ExternalInput")
    cc = nc.dram_tensor("cc", [t_len, N], f32, kind="ExternalInput")
    ss = nc.dram_tensor("ss", [t_len, N], f32, kind="ExternalInput")
    mu = nc.dram_tensor("mu", [128, 256], f32, kind="ExternalInput")
    ident = nc.dram_tensor("ident", [128, 128], bf, kind="ExternalInput")
    o = nc.dram_tensor("o", [hpc, t_len, N], f32, kind="ExternalOutput")

    with tile.TileContext(nc) as tc:
        with (
            tc.tile_pool(name="const", bufs=1) as constp,
            tc.tile_pool(name="head", bufs=2) as headp,
            tc.tile_pool(name="rope", bufs=3) as ropep,
            tc.tile_pool(name="work", bufs=3) as workp,
            tc.tile_pool(name="ps", bufs=2, space="PSUM") as psp,
            tc.tile_pool(name="psm", bufs=2, space="PSUM") as psmp,
        ):
            cc_sb = constp.tile([128, ch * N], f32)
            ss_sb = constp.tile([128, ch * N], f32)
            mu_sb = constp.tile([128, 256], f32)  # [mask | mask] for pairs
            id_sb = constp.tile([128, 128], bf)
            nc.sync.dma_start(
                cc_sb.rearrange("p (c n) -> p c n", c=ch),
                cc.rearrange("(c p) n -> p c n", p=128),
            )
            nc.sync.dma_start(
                ss_sb.rearrange("p (c n) -> p c n", c=ch),
                ss.rearrange("(c p) n -> p c n", p=128),
            )
            nc.sync.dma_start(mu_sb[:], mu[:])
            nc.sync.dma_start(id_sb[:], ident[:])

            for h in range(hpc):
                q_sb = headp.tile([128, ch * N], f32, tag="q")
                v32 = headp.tile([128, ch * N], f32, tag="v32")
                v_sb = headp.tile([128, ch * N], bf, tag="v")
                qr = headp.tile([128, ch * N], bf, tag="qr")
                qrt = headp.tile([64, ch * 128], bf, tag="qrt")

                q3 = q_sb.rearrange("p (c n) -> p c n", c=ch)
                v3 = v32.rearrange("p (c n) -> p c n", c=ch)

                # load + rope, pipelined per quarter-head
                for qt in range(nq):
                    csl = slice(qt * qtr, (qt + 1) * qtr)
                    fsl = slice(qt * qtr * N, (qt + 1) * qtr * N)
                    nc.sync.dma_start(q3[:, csl], q[h].rearrange(
                        "(c p) n -> p c n", p=128)[:, csl])
                    # HWDGE f32 load (SWDGE cast-DMA is ~10x slower), then
                    # a DVE cast to bf16
                    nc.sync.dma_start(v3[:, csl], v[h].rearrange(
                        "(c p) n -> p c n", p=128)[:, csl])
                    nc.vector.tensor_copy(v_sb[:, fsl], v32[:, fsl])

                    # swap(Q): exchange feature pairs, cast to bf16 (GpSimd)
                    swp = ropep.tile([128, qtr * N], bf, tag="swp")
                    sw4 = swp.rearrange("p (c m o) -> p c m o", c=qtr, m=32, o=2)
                    q4 = q3[:, csl].rearrange("p c (m o) -> p c m o", m=32)
                    nc.gpsimd.tensor_copy(sw4[:, :, :, 0], q4[:, :, :, 1])
                    nc.gpsimd.tensor_copy(sw4[:, :, :, 1], q4[:, :, :, 0])

                    # QR = Q*CC + swap(Q)*SS   (contiguous DVE ops)
                    t1 = ropep.tile([128, qtr * N], f32, tag="t1")
                    t2 = ropep.tile([128, qtr * N], f32, tag="t2")
                    nc.vector.tensor_mul(t1[:], q_sb[:, fsl], cc_sb[:, fsl])
                    nc.vector.tensor_mul(t2[:], swp[:], ss_sb[:, fsl])
                    nc.vector.tensor_add(qr[:, fsl], t1[:], t2[:])

                    # QR^T strips, grouped here so transpose-mode switches
                    # don't flush the PE pipeline inside the matmul stream
                    for cp in range(qt * qtr // 2, (qt + 1) * qtr // 2):
                        c0, c1 = 2 * cp, 2 * cp + 1
                        tr_ps = psp.tile([64, 256], bf, tag="tr")
                        nc.tensor.transpose(
                            tr_ps[:, 0:128], qr[:, c0 * 64:(c0 + 1) * 64],
                            id_sb[:],
                        )
                        nc.tensor.transpose(
                            tr_ps[:, 128:256], qr[:, c1 * 64:(c1 + 1) * 64],
                            id_sb[:],
                        )
                        nc.scalar.copy(
                            qrt[:, cp * 256:(cp + 1) * 256], tr_ps[:]
                        )

                m_ps = psmp.tile([128, 64], f32, tag="m")  # fp32 state
                mb_prev = None
                ost = None
                for cp in range(ch // 2):
                    c0, c1 = 2 * cp, 2 * cp + 1

                    # intra: P blocks for both chunks into one PSUM tile,
                    # one masked copy (strict-upper mask doubled)
                    p_ps = psp.tile([128, 256], f32, tag="p")
                    p_sb = workp.tile([128, 256], bf, tag="psb")
                    out_ps = psp.tile([128, 128], f32, tag="out")
                    for k, c in ((0, c0), (1, c1)):
                        qrt_c = qrt[:, c * 128:(c + 1) * 128]
                        nc.tensor.matmul(
                            p_ps[:, k * 128:(k + 1) * 128], qrt_c, qrt_c,
                            start=True, stop=True,
                        )
                    nc.vector.tensor_mul(p_sb[:], p_ps[:], mu_sb[:])

                    for k, c in ((0, c0), (1, c1)):
                        qrt_c = qrt[:, c * 128:(c + 1) * 128]
                        v_c = v_sb[:, c * 64:(c + 1) * 64]
                        qr_c = qr[:, c * 64:(c + 1) * 64]
                        osl = slice(k * 64, (k + 1) * 64)
                        if c == 0:
                            nc.tensor.matmul(
                                out_ps[:, osl], p_sb[:, k * 128:(k + 1) * 128],
                                v_c, start=True, stop=True,
                            )
                        else:
                            # inter: out += QR_c @ M  (M = state after c-1)
                            nc.tensor.matmul(
                                out_ps[:, osl], qrt_c, mb_prev[0:64, :],
                                start=True, stop=False,
                            )
                            nc.tensor.matmul(
                                out_ps[:, osl], p_sb[:, k * 128:(k + 1) * 128],
                                v_c, start=False, stop=True,
                            )

                        # state: M += QR_c^T @ V_c, accumulated in PSUM
                        nc.tensor.matmul(
                            m_ps[0:64, :], qr_c, v_c,
                            start=(c == 0), stop=(c == ch - 1),
                            skip_group_check=True,
                        )
                        if c < ch - 1:
                            m_bf = workp.tile([64, 64], bf, tag="mbf")
                            nc.scalar.copy(m_bf[:], m_ps[0:64, :])
                            mb_prev = m_bf

                    # batch output: stage 4 chunks (2 pairs), then one DMA
                    k2 = cp % 2
                    if k2 == 0:
                        ost = workp.tile([128, 256], f32, tag="ost")
                    nc.scalar.copy(ost[:, k2 * 128:(k2 + 1) * 128], out_ps[:])
                    if k2 == 1:
                        g4 = cp // 2 * 4
                        nc.sync.dma_start(
                            o[h].rearrange("(g p) n -> p g n", p=128)[
                                :, g4:g4 + 4],
                            ost.rearrange("p (g n) -> p g n", g=4),
                        )

    nc.compile()
    return nc


_CACHE = {}


def _get_program():
    if "nc" not in _CACHE:
        _CACHE["nc"] = build_program()
    return _CACHE["nc"]


def _strict_upper_mask():
    # lhsT for the diag block: keep P[j, i] where j < i; doubled for pairs
    m = np.triu(np.ones((128, 128), dtype=np.float32), k=1)
    return np.ascontiguousarray(np.concatenate([m, m], axis=1))


def _identity():
    import ml_dtypes

    return np.eye(128, dtype=ml_dtypes.bfloat16)


def kernel(Q, V):
    from concourse.bass_utils import run_bass_kernel_spmd

    Q = np.ascontiguousarray(np.asarray(Q), dtype=np.float32)
    V = np.ascontiguousarray(np.asarray(V), dtype=np.float32)
    qf = Q.reshape(NCORES, HPC, T, N)
    vf = V.reshape(NCORES, HPC, T, N)
    cc, ss = _host_tables(T)
    mu = _strict_upper_mask()
    ident = _identity()

    nc = _get_program()
    in_maps = [
        {"q": qf[i], "v": vf[i], "cc": cc, "ss": ss, "mu": mu, "ident": ident}
        for i in range(NCORES)
    ]
    res = run_bass_kernel_spmd(nc, in_maps, core_ids=list(range(NCORES)))
    out = np.stack([r["o"] for r in res.results], axis=0)
    return out.reshape(B, H, T, N)


# revision 43
# speedup vs baseline: 1.1782x; 1.1782x over previous
"""Trainium2 Bass kernel for nn_Attention_23424751632639.

Computation (per (b,h)):  out = tril_strict(rope(Q) @ rope(Q).T / sqrt(N)) @ V
Reformulated as chunked linear attention (exact, just reordered sums):
  out_c = QR_c @ M_c  +  strict_mask(QR_c @ QR_c^T) @ V_c
  M_{c+1} = M_c + QR_c^T @ V_c            (M is the [64,64] running state)
with QR = rope(Q) * N**-0.25 (scale folded into the cos/sin tables, so the
score scale N**-0.5 appears automatically in both the intra and inter terms).

RoPE is computed as  QR = Q*CC + swap(Q)*SS  where swap exchanges feature
pairs (2m <-> 2m+1) and the rotation sign is folded into SS.  swap runs on
GpSimd (otherwise idle), the three wide elementwise ops on DVE.

Matmul operands are bf16 (PE 1 cyc/row vs 4 for fp32); all accumulation
(PSUM, the M state) stays fp32.  QR^T strips come from PE transposes.
Two heads are processed interleaved at chunk-pair granularity so the serial
state -> M-cast -> inter chain of one head hides behind the other head's
matmuls; chunks are paired in PSUM so DVE/ACT fixups run at double width.

Sharding: B*H = 32 (b,h) pairs -> 4 per core across 8 cores; no collectives.
"""

import math
import sys

import numpy as np

if "/opt/trn_rl_repo" not in sys.path:
    sys.path.insert(0, "/opt/trn_rl_repo")

B, H, T, N = 2, 16, 4096, 64
THETA = 2.0 ** 16
NCORES = 8
HPC = (B * H) // NCORES  # heads per core
QTR = 8  # chunks per pipeline stage (quarter-head granularity)


def _host_tables(t_len):
    """Full-width scaled RoPE tables CC, SS [t_len, N] float32."""
    n = np.arange(N, dtype=np.float64)
    tq = np.floor(n / 2.0) * 2.0
    freqs = 1.0 / (THETA ** (tq / N)) / (2.0 * math.pi)  # [N]
    t = np.arange(t_len, dtype=np.float64)[:, None]
    ang = ((t * freqs[None, :]) % 1.0) * (2.0 * math.pi)  # [t_len, N]
    scale = float(N) ** -0.25
    cc = (np.cos(ang) * scale).astype(np.float32)
    ss = (np.sin(ang) * scale).astype(np.float32)
    ss[:, 0::2] *= -1.0
    return np.ascontiguousarray(cc), np.ascontiguousarray(ss)


def build_program(t_len=T, hpc=HPC):
    import concourse.mybir as mybir
    import concourse.tile as tile
    from concourse import bacc

    f32 = mybir.dt.float32
    bf = mybir.dt.bfloat16
    ch = t_len // 128  # number of 128-row chunks per head
    qtr = min(QTR, ch)  # chunks per pipeline stage
    nq = ch // qtr  # pipeline stages per head
    group = min(2, hpc)  # heads interleaved together

    nc = bacc.Bacc(None, target_bir_lowering=False)
    q = nc.dram_tensor("q", [hpc, t_len, N], f32, kind="ExternalInput")
    v = nc.dram_tensor("v", [hpc, t_len, N], f32, kind="ExternalInput")
    cc = nc.dram_tensor("cc", [t_len, N], f32, kind="ExternalInput")
    ss = nc.dram_tensor("ss", [t_len, N], f32, kind="ExternalInput")
    mu = nc.dram_tensor("mu", [128, 256], f32, kind="ExternalInput")
    ident = nc.dram_tensor("ident", [128, 128], bf, kind="ExternalInput")
    o = nc.dram_tensor("o", [hpc, t_len, N], f32, kind="ExternalOutput")

    with tile.TileContext(nc) as tc:
        with (
            tc.tile_pool(name="const", bufs=1) as constp,
            tc.tile_pool(name="head", bufs=3) as headp,
            tc.tile_pool(name="rope", bufs=3) as ropep,
            tc.tile_pool(name="work", bufs=3) as workp,
            tc.tile_pool(name="ps", bufs=2, space="PSUM") as psp,
            tc.tile_pool(name="psm", bufs=2, space="PSUM") as psmp,
        ):
            cc_sb = constp.tile([128, ch * N], f32)
            ss_sb = constp.tile([128, ch * N], f32)
            mu_sb = constp.tile([128, 256], f32)  # [mask | mask] for pairs
            id_sb = constp.tile([128, 128], bf)
            nc.sync.dma_start(
                cc_sb.rearrange("p (c n) -> p c n", c=ch),
                cc.rearrange("(c p) n -> p c n", p=128),
            )
            nc.sync.dma_start(
                ss_sb.rearrange("p (c n) -> p c n", c=ch),
                ss.rearrange("(c p) n -> p c n", p=128),
            )
            nc.sync.dma_start(mu_sb[:], mu[:])
            nc.sync.dma_start(id_sb[:], ident[:])

            for hg in range(hpc // group):
                heads = [hg * group + i for i in range(group)]
                qr = {}
                qrt = {}
                v_sb = {}
                m_ps = {}
                mb_prev = {}
                ost = {}

                # load + rope + transposes, pipelined per quarter-head
                for qt in range(nq):
                    csl = slice(qt * qtr, (qt + 1) * qtr)
                    fsl = slice(qt * qtr * N, (qt + 1) * qtr * N)
                    for h in heads:
                        if qt == 0:
                            qr[h] = headp.tile([128, ch * N], bf, name=f"qr{h}",
                                               tag=f"qr{h % group}")
                            qrt[h] = headp.tile([64, ch * 128], bf, name=f"qrt{h}",
                                                tag=f"qrt{h % group}")
                            v_sb[h] = headp.tile([128, ch * N], bf, name=f"v{h}",
                                                 tag=f"v{h % group}")
                        q_sb = ropep.tile([128, qtr * N], f32, tag="q")
                        v32 = ropep.tile([128, qtr * N], f32, tag="v32")
                        nc.sync.dma_start(
                            q_sb.rearrange("p (c n) -> p c n", c=qtr),
                            q[h].rearrange("(c p) n -> p c n", p=128)[:, csl],
                        )
                        nc.sync.dma_start(
                            v32.rearrange("p (c n) -> p c n", c=qtr),
                            v[h].rearrange("(c p) n -> p c n", p=128)[:, csl],
                        )
                        nc.vector.tensor_copy(v_sb[h][:, fsl], v32[:])

                        # swap(Q): exchange feature pairs -> bf16 (GpSimd)
                        swp = ropep.tile([128, qtr * N], bf, tag="swp")
                        sw4 = swp.rearrange("p (c m o) -> p c m o",
                                            c=qtr, m=32, o=2)
                        q4 = q_sb.rearrange("p (c m o) -> p c m o",
                                            c=qtr, m=32, o=2)
                        nc.gpsimd.tensor_copy(sw4[:, :, :, 0], q4[:, :, :, 1])
                        nc.gpsimd.tensor_copy(sw4[:, :, :, 1], q4[:, :, :, 0])

                        # QR = Q*CC + swap(Q)*SS   (contiguous DVE ops)
                        t1 = ropep.tile([128, qtr * N], f32, tag="t1")
                        t2 = ropep.tile([128, qtr * N], f32, tag="t2")
                        nc.vector.tensor_mul(t1[:], q_sb[:], cc_sb[:, fsl])
                        nc.vector.tensor_mul(t2[:], swp[:], ss_sb[:, fsl])
                        nc.vector.tensor_add(qr[h][:, fsl], t1[:], t2[:])

                        # QR^T strips via PE transpose, one ACT copy per pair
                        for cp in range(qt * qtr // 2, (qt + 1) * qtr // 2):
                            c0, c1 = 2 * cp, 2 * cp + 1
                            tr_ps = psp.tile([64, 256], bf, tag="tr")
                            nc.tensor.transpose(
                                tr_ps[:, 0:128],
                                qr[h][:, c0 * 64:(c0 + 1) * 64], id_sb[:],
                            )
                            nc.tensor.transpose(
                                tr_ps[:, 128:256],
                                qr[h][:, c1 * 64:(c1 + 1) * 64], id_sb[:],
                            )
                            nc.scalar.copy(
                                qrt[h][:, cp * 256:(cp + 1) * 256], tr_ps[:]
                            )

                for h in heads:
                    m_ps[h] = psmp.tile([128, 64], f32, name=f"m{h}",
                                        tag=f"m{h % group}", bufs=1)

                for cp in range(ch // 2):
                    c0, c1 = 2 * cp, 2 * cp + 1
                    for h in heads:
                        # intra: P blocks for both chunks into one PSUM tile,
                        # one masked copy (strict-upper mask doubled)
                        p_ps = psp.tile([128, 256], f32, tag="p")
                        p_sb = workp.tile([128, 256], bf, tag="psb")
                        out_ps = psp.tile([128, 128], f32, tag="out")
                        for k, c in ((0, c0), (1, c1)):
                            qrt_c = qrt[h][:, c * 128:(c + 1) * 128]
                            nc.tensor.matmul(
                                p_ps[:, k * 128:(k + 1) * 128], qrt_c, qrt_c,
                                start=True, stop=True,
                            )
                        nc.vector.tensor_mul(p_sb[:], p_ps[:], mu_sb[:])

                        for k, c in ((0, c0), (1, c1)):
                            qrt_c = qrt[h][:, c * 128:(c + 1) * 128]
                            v_c = v_sb[h][:, c * 64:(c + 1) * 64]
                            qr_c = qr[h][:, c * 64:(c + 1) * 64]
                            osl = slice(k * 64, (k + 1) * 64)
                            if c == 0:
                                nc.tensor.matmul(
                                    out_ps[:, osl],
                                    p_sb[:, k * 128:(k + 1) * 128],
                                    v_c, start=True, stop=True,
                                )
                            else:
                                # inter: out += QR_c @ M (state after c-1)
                                nc.tensor.matmul(
                                    out_ps[:, osl], qrt_c,
                                    mb_prev[h][0:64, :],
                                    start=True, stop=False,
                                )
                                nc.tensor.matmul(
                                    out_ps[:, osl],
                                    p_sb[:, k * 128:(k + 1) * 128],
                                    v_c, start=False, stop=True,
                                )

                            # state: M += QR_c^T @ V_c, accumulated in PSUM
                            nc.tensor.matmul(
                                m_ps[h][0:64, :], qr_c, v_c,
                                start=(c == 0), stop=(c == ch - 1),
                                skip_group_check=True,
                            )
                            if c < ch - 1:
                                m_bf = workp.tile([128, 64], bf,
                                                  tag=f"mbf{h % group}")
                                nc.scalar.copy(m_bf[0:64, :], m_ps[h][0:64, :])
                                mb_prev[h] = m_bf

                        # batch output: stage 4 chunks (2 pairs), one DMA
                        k2 = cp % 2
                        if k2 == 0:
                            ost[h] = workp.tile([128, 256], f32, name=f"ost{h}",
                                                tag=f"ost{h % group}")
                        nc.scalar.copy(
                            ost[h][:, k2 * 128:(k2 + 1) * 128], out_ps[:]
                        )
                        if k2 == 1:
                            g4 = cp // 2 * 4
                            nc.sync.dma_start(
                                o[h].rearrange("(g p) n -> p g n", p=128)[
                                    :, g4:g4 + 4],
                                ost[h].rearrange("p (g n) -> p g n", g=4),
                            )

    nc.compile()
    return nc


_CACHE = {}


def _get_program():
    if "nc" not in _CACHE:
        _CACHE["nc"] = build_program()
    return _CACHE["nc"]


def _strict_upper_mask():
    # lhsT for the diag block: keep P[j, i] where j < i; doubled for pairs
    m = np.triu(np.ones((128, 128), dtype=np.float32), k=1)
    return np.ascontiguousarray(np.concatenate([m, m], axis=1))


def _identity():
    import ml_dtypes

    return np.eye(128, dtype=ml_dtypes.bfloat16)


def kernel(Q, V):
    from concourse.bass_utils import run_bass_kernel_spmd

    Q = np.ascontiguousarray(np.asarray(Q), dtype=np.float32)
    V = np.ascontiguousarray(np.asarray(V), dtype=np.float32)
    qf = Q.reshape(NCORES, HPC, T, N)
    vf = V.reshape(NCORES, HPC, T, N)
    cc, ss = _host_tables(T)
    mu = _strict_upper_mask()
    ident = _identity()

    nc = _get_program()
    in_maps = [
        {"q": qf[i], "v": vf[i], "cc": cc, "ss": ss, "mu": mu, "ident": ident}
        for i in range(NCORES)
    ]
    res = run_bass_kernel_spmd(nc, in_maps, core_ids=list(range(NCORES)))
    out = np.stack([r["o"] for r in res.results], axis=0)
    return out.reshape(B, H, T, N)
